# revision 9
# baseline (speedup 1.0000x reference)
"""LoRA ColumnParallelLinear on 8 trn2 NeuronCores.

Math: reference returns (x @ W^T + 2.0 * (x @ A^T) @ B^T, bias).
Since (x @ A^T) @ B^T == x @ (B @ A)^T, we fold the LoRA branch into the
weight on the host:  W_eff = W + 2.0 * B @ A, so the device kernel is a
single GEMM  out = x @ W_eff^T.

Sharding (column-parallel): W_eff is split along the output dim O=16384
into 8 shards of 2048 rows; x is replicated. Each core computes
out_s = x @ W_eff_s^T -> [8192, 2048]; host concatenates along O.

Device layout: contraction dim D=4096 must sit on SBUF partitions, so the
host pre-transposes both operands into [p=128, ko=32, free] (bf16), where
D = ko*128 + p. Per token-tile m (128 tokens), PSUM accumulates over the
32 k-tiles into 4 banks of [128, 512] fp32, which are copied back and
DMA'd out as fp32.
"""

import sys
import types

import numpy as np
import ml_dtypes

# This container ships only a stub `antenv` package; bass_utils imports
# antenv.axon_hooks when tracing is requested (e.g. BASS_TRACE=1 in the
# env), which would raise ModuleNotFoundError. Register a graceful stub
# (hook absent -> bass_utils logs a warning and runs without a trace).
try:
    import antenv.axon_hooks  # noqa: F401
except ImportError:
    _stub = types.ModuleType("antenv.axon_hooks")
    _stub.get_axon_ntff_profile_hook = lambda: None
    sys.modules["antenv.axon_hooks"] = _stub

import concourse.bass as bass
import concourse.mybir as mybir
import concourse.tile as tile
from concourse import bacc
from concourse.bass_utils import run_bass_kernel_spmd

BF16 = mybir.dt.bfloat16
FP32 = mybir.dt.float32

B, S, D, O = 2, 2048, 4096, 16384
T = B * S                     # 8192 tokens
NCORES = 8
O_S = O // NCORES             # 2048 output features per core
KO = D // 128                 # 32 k-tiles
MT = T // 128                 # 64 token tiles
NT = O_S // 512               # 4 psum banks per token tile

_cache: dict = {}


def _build_nc(reps: int = 1) -> bass.Bass:
    # Bacc (not raw Bass): its compile() runs generate_event_semaphores,
    # which splits multi-wait instructions down to the TRN2 limit of one
    # wait each — walrus rejects DMACopies with >2 sync waits otherwise.
    # reps>1 repeats the whole GEMM inside one NEFF (benchmarking only:
    # the R=2 minus R=1 steady-state delta isolates pure HW time from
    # per-dispatch overhead).
    nc = bacc.Bacc()
    xT = nc.declare_dram_parameter("xT", [128, KO, T], BF16, isOutput=False)
    wT = nc.declare_dram_parameter("wT", [128, KO, O_S], BF16, isOutput=False)
    out = nc.declare_dram_parameter("out", [T, O_S], FP32, isOutput=True)

    with tile.TileContext(nc) as tc:
        with (
            tc.tile_pool(name="wpool", bufs=1) as wpool,
            tc.tile_pool(name="xpool", bufs=3) as xpool,
            tc.tile_pool(name="opool", bufs=3) as opool,
            tc.tile_pool(name="pspool", bufs=8, space="PSUM") as pspool,
        ):
            # Whole weight shard resident in SBUF: 32*2048*2B = 128 KiB/partition.
            # One tile per k-chunk (single writer each — walrus caps the
            # number of sync-waits it will put on one DMACopy, and 32 writes
            # into slices of a single tensor blow that cap).
            w_tiles = []
            for k in range(KO):
                w_k = wpool.tile([128, O_S], BF16, name=f"w_{k}")
                nc.sync.dma_start(out=w_k[:], in_=wT[:, k, :])
                w_tiles.append(w_k)

            for m in [m for _ in range(reps) for m in range(MT)]:
                x_sb = xpool.tile([128, KO, 128], BF16, name="x_sb")
                nc.sync.dma_start(out=x_sb[:], in_=xT[:, :, m * 128:(m + 1) * 128])

                ps = [
                    pspool.tile([128, 512], FP32, name="ps")
                    for _ in range(NT)
                ]
                for k in range(KO):
                    for n in range(NT):
                        nc.tensor.matmul(
                            ps[n][:],
                            x_sb[:, k, :],
                            w_tiles[k][:, n * 512:(n + 1) * 512],
                            start=(k == 0),
                            stop=(k == KO - 1),
                        )

                o_sb = opool.tile([128, O_S], FP32, name="o_sb")
                for n in range(NT):
                    nc.vector.tensor_copy(o_sb[:, n * 512:(n + 1) * 512], ps[n][:])
                nc.sync.dma_start(out=out[m * 128:(m + 1) * 128, :], in_=o_sb[:])

    nc.compile()
    return nc


def _get_nc() -> bass.Bass:
    if "nc" not in _cache:
        _cache["nc"] = _build_nc()
    return _cache["nc"]


def _prep_inputs(input_, weight, lora_A, lora_B):
    x = np.asarray(input_, dtype=np.float32).reshape(T, D)
    w_eff = np.asarray(weight, dtype=np.float32) + 2.0 * (
        np.asarray(lora_B, dtype=np.float32) @ np.asarray(lora_A, dtype=np.float32)
    )
    bf = ml_dtypes.bfloat16
    # x[t, ko*128+p] -> xT[p, ko, t]
    xT = np.ascontiguousarray(
        x.astype(bf).reshape(T, KO, 128).transpose(2, 1, 0)
    )
    in_maps = []
    for c in range(NCORES):
        ws = w_eff[c * O_S:(c + 1) * O_S].astype(bf)  # [2048, 4096]
        # ws[n, ko*128+p] -> wT[p, ko, n]
        wT = np.ascontiguousarray(ws.reshape(O_S, KO, 128).transpose(2, 1, 0))
        in_maps.append({"xT": xT, "wT": wT})
    return in_maps


def _run(input_, weight, lora_A, lora_B, trace=False):
    in_maps = _prep_inputs(input_, weight, lora_A, lora_B)
    bkr = run_bass_kernel_spmd(_get_nc(), in_maps, list(range(NCORES)), trace=trace)
    _cache["last_bkr"] = bkr
    out = np.concatenate(
        [bkr.results[c]["out"] for c in range(NCORES)], axis=1
    )  # [8192, 16384] fp32
    return out.reshape(B, S, O)


def kernel(input_, weight, bias, lora_A, lora_B):
    out = _run(input_, weight, lora_A, lora_B, trace=False)
    return (out, np.asarray(bias, dtype=np.float32))
